# revision 15
# baseline (speedup 1.0000x reference)
"""Per-example LoRA delta: out[b] = (x[b] @ A[b].T) @ B[b].T

Shapes (hardcoded): x [64, 2048, 512] bf16, A [64, 8, 512] bf16,
B [64, 1024, 8] bf16 -> out [64, 2048, 1024] bf16.

Strategy: batch-parallel across 8 NeuronCores (8 examples per core, zero
communication). Per core the problem is memory-bound (~48 MiB HBM traffic
vs ~0.4 GFLOP), so the kernel is organized around the two HBM streams:
  - Inputs are pre-laid-out on host (x transposed to [ex, i, s]; A, B
    transposed per example) so every device DMA is a big contiguous
    transfer: 1 MiB x^T loads on the SP HWDGE ring, 1 MiB output stores
    via SWDGE (gpsimd) — two independent queues that overlap.
  - mm1 (M=8): col-tiled 4-wide with tile_position=(0,32j) — the four
    s-chunks of an example compute concurrently into four partition strips
    of one PSUM bank, so one DVE copy moves the whole example's hT.
  - mm2 (K=8): row-tiled 4-wide with tile_position=(32j,0) — four s-chunks
    stream concurrently through disjoint 32-row strips of the PE array,
    two 512-wide matmuls filling a 2-bank [128,1024] PSUM tile.
  - PSUM->SBUF f32->bf16 casts (the engine bottleneck after HBM) run
    [128,1024]-wide to amortize fixed costs, split ~45/55 between the
    Vector and Scalar engines.
"""

import numpy as np

import concourse.bass as bass  # noqa: F401  (engine namespaces come via nc)
import concourse.mybir as mybir
import concourse.tile as tile
from concourse import bacc
from concourse.bass_utils import run_bass_kernel_spmd

BSZ, SEQ, D_IN, D_OUT, RANK = 64, 2048, 512, 1024, 8
N_CORES = 8
EX = BSZ // N_CORES          # 8 examples per core
SC = 512                     # s-chunk (PSUM-bank-sized matmul free dim)
NSC = SEQ // SC              # 4 s-chunks per example
IC = 128                     # contraction chunk (partition dim)
NIC = D_IN // IC             # 4
GE = 2                       # examples per x^T load group (1 MiB DMAs)
NG = EX // GE                # 4 groups
NSUB = SC // 128             # 4 output row-blocks per s-chunk
NOC = D_OUT // 512           # 2 matmul col-chunks per PSUM tile

BF16 = mybir.dt.bfloat16
F32 = mybir.dt.float32

_CACHE = {}


def _build():
    nc = bacc.Bacc(
        "TRN2", target_bir_lowering=False, debug=False, num_devices=N_CORES
    )
    xt_ap = nc.dram_tensor("xt", [EX, D_IN, SEQ], BF16, kind="ExternalInput").ap()
    at_ap = nc.dram_tensor("at", [EX, D_IN, RANK], BF16, kind="ExternalInput").ap()
    bt_ap = nc.dram_tensor("bt", [EX, RANK, D_OUT], BF16, kind="ExternalInput").ap()
    y_ap = nc.dram_tensor("y", [EX * SEQ, D_OUT], BF16, kind="ExternalOutput").ap()

    y_r = y_ap.rearrange(
        "(ex j u p) o -> ex j p u o", ex=EX, j=NSC, u=NSUB, p=128
    )
    xt_r = xt_ap.rearrange("e i s -> i e s")

    with tile.TileContext(nc) as tc:
        with (
            tc.tile_pool(name="const", bufs=1) as const_pool,
            tc.tile_pool(name="xt", bufs=2) as xt_pool,
            tc.tile_pool(name="htp", bufs=2, space="PSUM") as ht_psum,
            tc.tile_pool(name="ht", bufs=2) as ht_pool,
            tc.tile_pool(name="dp", bufs=6, space="PSUM") as d_psum,
            tc.tile_pool(name="out", bufs=2) as out_pool,
        ):
            # A^T for all examples: [p=128, ex, c, r]
            at_sb = const_pool.tile([128, EX, NIC, RANK], BF16)
            nc.sync.dma_start(
                at_sb[:], at_ap.rearrange("ex (c p) r -> p ex c r", p=128)
            )

            # PE warmup: dummy matmuls keep the PE busy through the pipeline
            # fill so the HAM clock gate flips to 8/8 (2.4 GHz) and stays
            # there — an idle gap >~3.4us would re-throttle it to 1.2 GHz
            # for the rest of the kernel.
            wtile = const_pool.tile([128, 512], BF16)
            nc.gpsimd.memset(wtile[:], 0)
            wpsum = ht_psum.tile([128, SC], F32, name="wpsum", tag="pht")
            for _ in range(36):
                nc.tensor.matmul(
                    wpsum[:], wtile[:, :128], wtile[:], start=True, stop=True
                )

            # B^T for all examples, replicated at 4 partition strips for
            # row-tiled mm2: partitions 32j..32j+8 hold [r, ex, o].
            # Issued on the scalar HWDGE ring so the sync ring reaches the
            # first x^T load immediately (mm2 needs B^T later than mm1
            # needs x^T).
            bt_sb = const_pool.tile([128, EX, D_OUT], BF16)

            copy_idx = 0
            for g in range(NG):
                # x^T loads for GE examples, one i-chunk per DMA (1 MiB each).
                # Group 0 is the pipeline fill: split per-example and across
                # both HWDGE rings, example-0 chunks first, so mm1 can start
                # as early as possible; B^T rides the scalar ring behind the
                # e0 chunks. Later groups are prefetch-covered.
                xts = []
                if g == 0:
                    for c in range(NIC):
                        xts.append(xt_pool.tile([128, GE, SEQ], BF16, tag=f"xt{c}", name=f"xt{c}_0"))
                    for e in range(GE):
                        for c in range(NIC):
                            eng = nc.scalar if c % 2 == 1 else nc.sync
                            eng.dma_start(
                                xts[c][:, e, :],
                                xt_r[c * IC : (c + 1) * IC, e, :],
                            )
                        if e == 0:
                            for j in range(4):
                                nc.scalar.dma_start(
                                    bt_sb[32 * j : 32 * j + RANK],
                                    bt_ap.rearrange("ex r o -> r ex o"),
                                )
                else:
                    for c in range(NIC):
                        t = xt_pool.tile([128, GE, SEQ], BF16, tag=f"xt{c}")
                        nc.sync.dma_start(
                            t[:], xt_r[c * IC : (c + 1) * IC, g * GE : (g + 1) * GE, :]
                        )
                        xts.append(t)

                for e in range(GE):
                    ex = g * GE + e
                    # --- mm1: hT for s-chunk j -> partition strip 32j of one
                    # PSUM bank, 4 strips computing concurrently (col-tiled).
                    pht = ht_psum.tile([128, SC], F32)
                    for c in range(NIC):
                        for j in range(NSC):
                            nc.tensor.matmul(
                                pht[32 * j : 32 * j + RANK, :],
                                at_sb[:, ex, c, :],
                                xts[c][:, e, j * SC : (j + 1) * SC],
                                start=(c == 0),
                                stop=(c == NIC - 1),
                                tile_position=(0, 32 * j),
                            )
                    ht = ht_pool.tile([128, SC], BF16)
                    nc.vector.tensor_copy(
                        ht[: 32 * 3 + RANK, :], pht[: 32 * 3 + RANK, :]
                    )

                    # --- mm2 row-tiled 4-wide: strip j = s-chunk j ---
                    ots = [
                        out_pool.tile(
                            [128, NSUB, D_OUT], BF16, tag=f"ot{j}", name=f"ot{j}_{ex}"
                        )
                        for j in range(NSC)
                    ]
                    for u in range(NSUB):
                        for k in range(NOC):
                            pds = []
                            for j in range(NSC):
                                pd = d_psum.tile([128, 512], F32, name="pd")
                                nc.tensor.matmul(
                                    pd[:],
                                    ht[32 * j : 32 * j + RANK, u * 128 : (u + 1) * 128],
                                    bt_sb[32 * j : 32 * j + RANK, ex, k * 512 : (k + 1) * 512],
                                    start=True,
                                    stop=True,
                                    tile_position=(32 * j, 0),
                                )
                                pds.append(pd)
                            for j in range(NSC):
                                dst = ots[j][:, u, k * 512 : (k + 1) * 512]
                                # split casts ~44/56 across DVE / ACT
                                if copy_idx % 9 < 4:
                                    nc.vector.tensor_copy(dst, pds[j][:])
                                else:
                                    nc.scalar.copy(dst, pds[j][:])
                                copy_idx += 1
                        # store completed halves early to smooth the
                        # SWDGE store stream and shorten the kernel tail
                        if u % 2 == 1:
                            for j in range(NSC):
                                nc.gpsimd.dma_start(
                                    y_r[ex, j][:, u - 1 : u + 1, :],
                                    ots[j][:, u - 1 : u + 1, :],
                                )

    nc.compile()
    return nc


def _get_nc():
    if "nc" not in _CACHE:
        _CACHE["nc"] = _build()
    return _CACHE["nc"]


def make_in_maps(x, A, B):
    x = np.asarray(x)
    A = np.asarray(A)
    B = np.asarray(B)
    in_maps = []
    for c in range(N_CORES):
        sl = slice(c * EX, (c + 1) * EX)
        in_maps.append(
            {
                "xt": np.ascontiguousarray(np.transpose(x[sl], (0, 2, 1))),
                "at": np.ascontiguousarray(np.transpose(A[sl], (0, 2, 1))),
                "bt": np.ascontiguousarray(np.transpose(B[sl], (0, 2, 1))),
            }
        )
    return in_maps


def kernel(x, A, B):
    in_maps = make_in_maps(x, A, B)
    nc = _get_nc()
    res = run_bass_kernel_spmd(nc, in_maps, list(range(N_CORES)))
    out = np.concatenate(
        [res.results[c]["y"].reshape(EX, SEQ, D_OUT) for c in range(N_CORES)],
        axis=0,
    )
    return out
